# revision 22
# baseline (speedup 1.0000x reference)
"""AdaptiveCenterLoss on 8 TRN2 NeuronCores.

loss = mean_i ||features[i] - centers[labels[i]]||^2
     with B=131072, D=256, C=1000.

Strategy (data-parallel, memory-bound):
  - host-side, sort rows by label and pack them into 16-row blocks, each
    block sharing one label; partial blocks are padded with rows equal to
    that class's center (contributing exactly 0 to the sum)
  - shard the padded blocks across 8 cores x 128 partitions; per tile
    (one block per partition), ONE [128,1]-index indirect DMA gathers the
    128 needed center rows (the per-descriptor DGE cost is the bottleneck
    of any per-row gather on this HW, so one gather per 16-row block
    instead of per row is ~16x cheaper)
  - per tile, either
      MODE 'sub':   DVE subtract (center broadcast across the 16 slots via
                    a stride-0 AP) then ACT square + row-sum accumulate
      MODE 'terms': sum ||f||^2 (ACT) and sum_s f (DVE reduce) in parallel
                    straight off the feature tile, then tiny per-tile ops
                    for the cross term and S*||c||^2; host combines
  - each core outputs per-tile partial sums; host sums and divides by B
"""

import numpy as np

import concourse.bacc as bacc
import concourse.bass as bass
import concourse.mybir as mybir
import concourse.tile as tile
from concourse.bass_utils import run_bass_kernel_spmd

B, D, C = 131072, 256, 1000
N_CORES = 8
P = 128   # SBUF partitions
S = 16    # rows (slots) per block

MODE = "terms"  # 'sub' or 'terms'

_nc_cache = {}


def _build(nt):
    """Build the per-core graph for `nt` tiles (nt blocks per partition)."""
    key = (nt, MODE)
    if key in _nc_cache:
        return _nc_cache[key]
    nc = bacc.Bacc()
    feats = nc.declare_dram_parameter(
        "features", [nt * P * S, D], mybir.dt.float32, isOutput=False
    )
    labels = nc.declare_dram_parameter("labels", [P, nt], mybir.dt.int32, isOutput=False)
    centers = nc.declare_dram_parameter(
        "centers", [C, D], mybir.dt.float32, isOutput=False
    )
    out_cols = nt if MODE == "sub" else 3 * nt
    out = nc.declare_dram_parameter(
        "out", [P, out_cols], mybir.dt.float32, isOutput=True
    )

    # tile t, partition p, slot s <- padded feature row (t*128 + p)*S + s
    fview = feats[:].rearrange("(t p s) d -> t p s d", p=P, s=S)

    with tile.TileContext(nc) as tc:
        with (
            tc.tile_pool(name="lab", bufs=1) as lab_pool,
            tc.tile_pool(name="f", bufs=4) as f_pool,
            tc.tile_pool(name="c", bufs=4) as c_pool,
            tc.tile_pool(name="w", bufs=3) as w_pool,
            tc.tile_pool(name="v", bufs=3) as v_pool,
            tc.tile_pool(name="acc", bufs=1) as acc_pool,
        ):
            lab = lab_pool.tile([P, nt], mybir.dt.int32)
            nc.sync.dma_start(out=lab[:], in_=labels[:])
            acc = acc_pool.tile([P, out_cols], mybir.dt.float32)
            for t in range(nt):
                f_t = f_pool.tile([P, S * D], mybir.dt.float32)
                nc.sync.dma_start(
                    out=f_t[:].rearrange("p (s d) -> p s d", s=S), in_=fview[t]
                )
                c_s = c_pool.tile([P, D], mybir.dt.float32)
                nc.gpsimd.indirect_dma_start(
                    out=c_s[:],
                    out_offset=None,
                    in_=centers[:],
                    in_offset=bass.IndirectOffsetOnAxis(ap=lab[:, t : t + 1], axis=0),
                )
                if MODE == "sub":
                    c_b = (
                        c_s[:]
                        .rearrange("p (s d) -> p s d", s=1)
                        .to_broadcast([P, S, D])
                    )
                    nc.vector.tensor_tensor(
                        out=f_t[:].rearrange("p (s d) -> p s d", s=S),
                        in0=f_t[:].rearrange("p (s d) -> p s d", s=S),
                        in1=c_b,
                        op=mybir.AluOpType.subtract,
                    )
                    nc.scalar.activation(
                        out=f_t[:],
                        in_=f_t[:],
                        func=mybir.ActivationFunctionType.Square,
                        accum_out=acc[:, t : t + 1],
                    )
                else:
                    # sum_s ||f_s||^2 -> acc[:, 3t]   (ACT, independent of gather)
                    sqf = w_pool.tile([P, S * D], mybir.dt.float32, tag="sqf")
                    nc.scalar.activation(
                        out=sqf[:],
                        in_=f_t[:],
                        func=mybir.ActivationFunctionType.Square,
                        accum_out=acc[:, 3 * t : 3 * t + 1],
                    )
                    # sum_s f_s [P, D]                 (DVE, independent of gather)
                    sumf = v_pool.tile([P, D], mybir.dt.float32, tag="sumf")
                    nc.vector.tensor_reduce(
                        out=sumf[:],
                        in_=f_t[:].rearrange("p (s d) -> p d s", s=S),
                        axis=mybir.AxisListType.X,
                        op=mybir.AluOpType.add,
                    )
                    # S * ||c||^2 -> acc[:, 3t+1]  (scale=sqrt(S) exact for S=16)
                    c2 = v_pool.tile([P, D], mybir.dt.float32, tag="c2")
                    nc.scalar.activation(
                        out=c2[:],
                        in_=c_s[:],
                        func=mybir.ActivationFunctionType.Square,
                        scale=float(np.sqrt(S)),
                        accum_out=acc[:, 3 * t + 1 : 3 * t + 2],
                    )
                    # dot(sum_f, c) -> acc[:, 3t+2]
                    prod = v_pool.tile([P, D], mybir.dt.float32, tag="prod")
                    nc.vector.tensor_tensor(
                        out=prod[:], in0=sumf[:], in1=c_s[:], op=mybir.AluOpType.mult
                    )
                    nc.scalar.activation(
                        out=prod[:],
                        in_=prod[:],
                        func=mybir.ActivationFunctionType.Copy,
                        accum_out=acc[:, 3 * t + 2 : 3 * t + 3],
                    )
            nc.sync.dma_start(out=out[:], in_=acc[:])
    nc.finalize()
    _nc_cache[key] = nc
    return nc


def _prepare(features, centers, labels):
    """Sort rows by label into padded S-row blocks; returns per-core maps + nt."""
    features = np.ascontiguousarray(np.asarray(features), dtype=np.float32)
    centers = np.ascontiguousarray(np.asarray(centers), dtype=np.float32)
    labels = np.asarray(labels).astype(np.int32)

    counts = np.bincount(labels, minlength=C)          # [C]
    nblocks = -(-counts // S)                          # ceil(n_c / S) per class
    nb = int(nblocks.sum())
    group = N_CORES * P                                # blocks per tile across chip
    nb_pad = -(-nb // group) * group
    nt = nb_pad // group

    # block labels, in sorted-class order; pad blocks use class 0
    block_labels = np.zeros(nb_pad, dtype=np.int32)
    block_labels[:nb] = np.repeat(np.arange(C, dtype=np.int32), nblocks)

    # every padded slot starts as its block's center row -> contributes 0
    fpad = centers[block_labels].repeat(S, axis=0).reshape(nb_pad * S, D)

    # scatter the real rows into their slots
    order = np.argsort(labels)
    labels_sorted = labels[order]
    class_row_start = np.concatenate(([0], np.cumsum(counts)[:-1]))
    class_slot_start = S * np.concatenate(([0], np.cumsum(nblocks)[:-1]))
    rank = np.arange(B) - class_row_start[labels_sorted]
    dst = class_slot_start[labels_sorted] + rank
    fpad[dst] = features[order]

    rows_core = nt * P * S
    maps = []
    for k in range(N_CORES):
        fs = fpad[k * rows_core : (k + 1) * rows_core]
        # labW[p, t] = block_labels[(k*nt + t)*128 + p]
        lw = np.ascontiguousarray(
            block_labels[k * nt * P : (k + 1) * nt * P].reshape(nt, P).T
        )
        maps.append({"features": fs, "labels": lw, "centers": centers})
    return maps, nt


def _reduce_outs(results, nt):
    total = 0.0
    for r in results:
        o = np.asarray(r["out"]).astype(np.float64)
        if MODE == "sub":
            total += o.sum()
        else:
            o = o.reshape(P, nt, 3)
            total += o[:, :, 0].sum() + o[:, :, 1].sum() - 2.0 * o[:, :, 2].sum()
    return total


def run(features, centers, labels, trace=False):
    """Run on 8 cores; returns (loss_scalar, BassKernelResults)."""
    maps, nt = _prepare(features, centers, labels)
    nc = _build(nt)
    res = run_bass_kernel_spmd(
        nc, maps, core_ids=list(range(N_CORES)), trace=trace
    )
    return np.float32(_reduce_outs(res.results, nt) / B), res


def kernel(features, centers, labels):
    loss, _ = run(features, centers, labels)
    return loss


# revision 23
# speedup vs baseline: 1.1762x; 1.1762x over previous
"""AdaptiveCenterLoss on 8 TRN2 NeuronCores.

loss = mean_i ||features[i] - centers[labels[i]]||^2
     with B=131072, D=256, C=1000.

Strategy (data-parallel, memory-bound):
  - host-side, sort rows by label and pack them into 8-row blocks, each
    block sharing one label; partial blocks are padded with rows equal to
    that class's center (contributing exactly 0 to the sum)
  - shard the padded blocks across 8 cores x 128 partitions as J 8-row
    "block-slots" per partition; per block-slot, ONE [128,1]-index
    indirect DMA gathers the 128 needed center rows (per-descriptor DGE
    cost makes per-row gathers ~16x more expensive on this HW)
  - block-slots are processed in pairs (one feature DMA + one DVE
    subtract with the two centers broadcast via a stride-0 4D AP + one
    ACT square+row-sum accumulate per pair); a trailing odd slot runs
    as a half tile, which also drains the pipeline faster
  - each core outputs per-tile partial sums; host sums and divides by B
"""

import numpy as np

import concourse.bacc as bacc
import concourse.bass as bass
import concourse.mybir as mybir
import concourse.tile as tile
from concourse.bass_utils import run_bass_kernel_spmd

B, D, C = 131072, 256, 1000
N_CORES = 8
P = 128   # SBUF partitions
S = 8     # rows per block (one label per block)

_nc_cache = {}


def _build(J):
    """Per-core graph for J block-slots per partition (J*8 rows each)."""
    if J in _nc_cache:
        return _nc_cache[J]
    # block-slot groups: pairs, plus a trailing single if J is odd
    groups = [(2 * u, 2) for u in range(J // 2)]
    if J % 2:
        groups.append((J - 1, 1))

    nc = bacc.Bacc()
    feats = nc.declare_dram_parameter(
        "features", [J * P * S, D], mybir.dt.float32, isOutput=False
    )
    labels = nc.declare_dram_parameter("labels", [P, J], mybir.dt.int32, isOutput=False)
    centers = nc.declare_dram_parameter(
        "centers", [C, D], mybir.dt.float32, isOutput=False
    )
    out = nc.declare_dram_parameter(
        "out", [P, len(groups)], mybir.dt.float32, isOutput=True
    )

    # block-slot j, partition p, slot s <- feature row (j*128 + p)*8 + s
    fview = feats[:].rearrange("(j p s) d -> p j s d", p=P, s=S)

    with tile.TileContext(nc) as tc:
        with (
            tc.tile_pool(name="lab", bufs=1) as lab_pool,
            tc.tile_pool(name="f", bufs=4) as f_pool,
            tc.tile_pool(name="c", bufs=4) as c_pool,
            tc.tile_pool(name="acc", bufs=1) as acc_pool,
        ):
            lab = lab_pool.tile([P, J], mybir.dt.int32)
            nc.sync.dma_start(out=lab[:], in_=labels[:])
            acc = acc_pool.tile([P, len(groups)], mybir.dt.float32)
            for gi, (j0, m) in enumerate(groups):
                f_t = f_pool.tile([P, m * S * D], mybir.dt.float32, tag="f")
                nc.sync.dma_start(
                    out=f_t[:].rearrange("p (b s d) -> p b s d", b=m, s=S),
                    in_=fview[:, j0 : j0 + m, :, :],
                )
                c_g = c_pool.tile([P, m * D], mybir.dt.float32, tag="c")
                for b in range(m):
                    nc.gpsimd.indirect_dma_start(
                        out=c_g[:, b * D : (b + 1) * D],
                        out_offset=None,
                        in_=centers[:],
                        in_offset=bass.IndirectOffsetOnAxis(
                            ap=lab[:, j0 + b : j0 + b + 1], axis=0
                        ),
                    )
                c_b = (
                    c_g[:]
                    .rearrange("p (b s d) -> p b s d", b=m, s=1)
                    .to_broadcast([P, m, S, D])
                )
                nc.vector.tensor_tensor(
                    out=f_t[:].rearrange("p (b s d) -> p b s d", b=m, s=S),
                    in0=f_t[:].rearrange("p (b s d) -> p b s d", b=m, s=S),
                    in1=c_b,
                    op=mybir.AluOpType.subtract,
                )
                nc.scalar.activation(
                    out=f_t[:],
                    in_=f_t[:],
                    func=mybir.ActivationFunctionType.Square,
                    accum_out=acc[:, gi : gi + 1],
                )
            nc.sync.dma_start(out=out[:], in_=acc[:])
    nc.finalize()
    _nc_cache[J] = nc
    return nc


def _prepare(features, centers, labels):
    """Sort rows by label into padded S-row blocks; returns per-core maps + J."""
    features = np.ascontiguousarray(np.asarray(features), dtype=np.float32)
    centers = np.ascontiguousarray(np.asarray(centers), dtype=np.float32)
    labels = np.asarray(labels).astype(np.int32)

    counts = np.bincount(labels, minlength=C)          # [C]
    nblocks = -(-counts // S)                          # ceil(n_c / S) per class
    nb = int(nblocks.sum())
    group = N_CORES * P                                # blocks per slot across chip
    nb_pad = -(-nb // group) * group
    J = nb_pad // group                                # block-slots per partition

    # block labels, in sorted-class order; pad blocks use class 0
    block_labels = np.zeros(nb_pad, dtype=np.int32)
    block_labels[:nb] = np.repeat(np.arange(C, dtype=np.int32), nblocks)

    # every padded slot starts as its block's center row -> contributes 0
    fpad = centers[block_labels].repeat(S, axis=0).reshape(nb_pad * S, D)

    # scatter the real rows into their slots
    order = np.argsort(labels)
    labels_sorted = labels[order]
    class_row_start = np.concatenate(([0], np.cumsum(counts)[:-1]))
    class_slot_start = S * np.concatenate(([0], np.cumsum(nblocks)[:-1]))
    rank = np.arange(B) - class_row_start[labels_sorted]
    dst = class_slot_start[labels_sorted] + rank
    fpad[dst] = features[order]

    rows_core = J * P * S
    maps = []
    for k in range(N_CORES):
        fs = fpad[k * rows_core : (k + 1) * rows_core]
        # labW[p, j] = block_labels[(k*J + j)*128 + p]
        lw = np.ascontiguousarray(
            block_labels[k * J * P : (k + 1) * J * P].reshape(J, P).T
        )
        maps.append({"features": fs, "labels": lw, "centers": centers})
    return maps, J


def run(features, centers, labels, trace=False):
    """Run on 8 cores; returns (loss_scalar, BassKernelResults)."""
    maps, J = _prepare(features, centers, labels)
    nc = _build(J)
    res = run_bass_kernel_spmd(
        nc, maps, core_ids=list(range(N_CORES)), trace=trace
    )
    total = 0.0
    for r in res.results:
        total += float(np.asarray(r["out"]).astype(np.float64).sum())
    return np.float32(total / B), res


def kernel(features, centers, labels):
    loss, _ = run(features, centers, labels)
    return loss


# revision 27
# speedup vs baseline: 1.2928x; 1.0991x over previous
"""AdaptiveCenterLoss on 8 TRN2 NeuronCores.

loss = mean_i ||features[i] - centers[labels[i]]||^2
     with B=131072, D=256, C=1000.

Strategy (data-parallel, memory-bound):
  - host-side, sort rows by label and pack them into 8-row blocks, each
    block sharing one label; partial blocks are padded with rows equal to
    that class's center (contributing exactly 0 to the sum)
  - shard the padded blocks across 8 cores x 128 partitions as J 8-row
    "block-slots" per partition; per block-slot, ONE [128,1]-index
    indirect DMA gathers the 128 needed center rows (per-descriptor DGE
    cost makes per-row gathers ~16x more expensive on this HW)
  - block-slots are processed in pairs (one feature DMA + one DVE
    subtract with the two centers broadcast via a stride-0 4D AP + one
    ACT square+row-sum accumulate per pair); a trailing odd slot runs
    as a half tile, which also drains the pipeline faster
  - each core outputs per-tile partial sums; host sums and divides by B
"""

import numpy as np

import concourse.bacc as bacc
import concourse.bass as bass
import concourse.mybir as mybir
import concourse.tile as tile
from concourse.bass_utils import run_bass_kernel_spmd

B, D, C = 131072, 256, 1000
N_CORES = 8
P = 128   # SBUF partitions
S = 16    # rows per block (one label per block)

_nc_cache = {}


def _build(J):
    """Per-core graph for J block-slots per partition (J*8 rows each)."""
    if J in _nc_cache:
        return _nc_cache[J]
    # first/last tiles are computed in halves for faster ramp/drain
    splits = [2 if t in (0, J - 1) else 1 for t in range(J)]
    acc_cols = sum(splits)

    nc = bacc.Bacc()
    feats = nc.declare_dram_parameter(
        "features", [J * P * S, D], mybir.dt.float32, isOutput=False
    )
    labels = nc.declare_dram_parameter("labels", [P, J], mybir.dt.int32, isOutput=False)
    centers = nc.declare_dram_parameter(
        "centers", [C, D], mybir.dt.float32, isOutput=False
    )
    out = nc.declare_dram_parameter(
        "out", [P, acc_cols], mybir.dt.float32, isOutput=True
    )

    # block-slot j, partition p, slot s <- feature row (j*128 + p)*8 + s
    fview = feats[:].rearrange("(j p s) d -> p j s d", p=P, s=S)

    with tile.TileContext(nc) as tc:
        with (
            tc.tile_pool(name="lab", bufs=1) as lab_pool,
            tc.tile_pool(name="f", bufs=4) as f_pool,
            tc.tile_pool(name="c", bufs=6) as c_pool,
            tc.tile_pool(name="acc", bufs=1) as acc_pool,
        ):
            lab = lab_pool.tile([P, J], mybir.dt.int32)
            nc.sync.dma_start(out=lab[:], in_=labels[:])
            acc = acc_pool.tile([P, acc_cols], mybir.dt.float32)
            col = 0
            for t in range(J):
                H = splits[t]
                SH = S // H
                f_t = f_pool.tile([P, S * D], mybir.dt.float32, tag="f")
                for h in range(H):
                    nc.sync.dma_start(
                        out=f_t[:, h * SH * D : (h + 1) * SH * D].rearrange(
                            "p (s d) -> p s d", s=SH
                        ),
                        in_=fview[:, t, h * SH : (h + 1) * SH, :],
                    )
                c_s = c_pool.tile([P, D], mybir.dt.float32, tag="c")
                nc.gpsimd.indirect_dma_start(
                    out=c_s[:],
                    out_offset=None,
                    in_=centers[:],
                    in_offset=bass.IndirectOffsetOnAxis(ap=lab[:, t : t + 1], axis=0),
                )
                c_b = (
                    c_s[:]
                    .rearrange("p (s d) -> p s d", s=1)
                    .to_broadcast([P, SH, D])
                )
                for h in range(H):
                    fh = f_t[:, h * SH * D : (h + 1) * SH * D]
                    nc.vector.tensor_tensor(
                        out=fh.rearrange("p (s d) -> p s d", s=SH),
                        in0=fh.rearrange("p (s d) -> p s d", s=SH),
                        in1=c_b,
                        op=mybir.AluOpType.subtract,
                    )
                    nc.scalar.activation(
                        out=fh,
                        in_=fh,
                        func=mybir.ActivationFunctionType.Square,
                        accum_out=acc[:, col : col + 1],
                    )
                    col += 1
            nc.sync.dma_start(out=out[:], in_=acc[:])
    nc.finalize()
    _nc_cache[J] = nc
    return nc


def _prepare(features, centers, labels):
    """Sort rows by label into padded S-row blocks; returns per-core maps + J."""
    features = np.ascontiguousarray(np.asarray(features), dtype=np.float32)
    centers = np.ascontiguousarray(np.asarray(centers), dtype=np.float32)
    labels = np.asarray(labels).astype(np.int32)

    counts = np.bincount(labels, minlength=C)          # [C]
    nblocks = -(-counts // S)                          # ceil(n_c / S) per class
    nb = int(nblocks.sum())
    group = N_CORES * P                                # blocks per slot across chip
    nb_pad = -(-nb // group) * group
    J = nb_pad // group                                # block-slots per partition

    # block labels, in sorted-class order; pad blocks use class 0
    block_labels = np.zeros(nb_pad, dtype=np.int32)
    block_labels[:nb] = np.repeat(np.arange(C, dtype=np.int32), nblocks)

    # every padded slot starts as its block's center row -> contributes 0
    fpad = centers[block_labels].repeat(S, axis=0).reshape(nb_pad * S, D)

    # scatter the real rows into their slots
    order = np.argsort(labels)
    labels_sorted = labels[order]
    class_row_start = np.concatenate(([0], np.cumsum(counts)[:-1]))
    class_slot_start = S * np.concatenate(([0], np.cumsum(nblocks)[:-1]))
    rank = np.arange(B) - class_row_start[labels_sorted]
    dst = class_slot_start[labels_sorted] + rank
    fpad[dst] = features[order]

    rows_core = J * P * S
    maps = []
    for k in range(N_CORES):
        fs = fpad[k * rows_core : (k + 1) * rows_core]
        # labW[p, j] = block_labels[(k*J + j)*128 + p]
        lw = np.ascontiguousarray(
            block_labels[k * J * P : (k + 1) * J * P].reshape(J, P).T
        )
        maps.append({"features": fs, "labels": lw, "centers": centers})
    return maps, J


def run(features, centers, labels, trace=False):
    """Run on 8 cores; returns (loss_scalar, BassKernelResults)."""
    maps, J = _prepare(features, centers, labels)
    nc = _build(J)
    res = run_bass_kernel_spmd(
        nc, maps, core_ids=list(range(N_CORES)), trace=trace
    )
    total = 0.0
    for r in res.results:
        total += float(np.asarray(r["out"]).astype(np.float64).sum())
    return np.float32(total / B), res


def kernel(features, centers, labels):
    loss, _ = run(features, centers, labels)
    return loss


# revision 29
# speedup vs baseline: 1.4049x; 1.0867x over previous
"""AdaptiveCenterLoss on 8 TRN2 NeuronCores.

loss = mean_i ||features[i] - centers[labels[i]]||^2
     with B=131072, D=256, C=1000.

Strategy (data-parallel, memory-bound):
  - host-side, sort rows by label and pack them into 8-row blocks, each
    block sharing one label; partial blocks are padded with rows equal to
    that class's center (contributing exactly 0 to the sum)
  - shard the padded blocks across 8 cores x 128 partitions as J 8-row
    "block-slots" per partition; per block-slot, ONE [128,1]-index
    indirect DMA gathers the 128 needed center rows (per-descriptor DGE
    cost makes per-row gathers ~16x more expensive on this HW)
  - block-slots are processed in pairs (one feature DMA + one DVE
    subtract with the two centers broadcast via a stride-0 4D AP + one
    ACT square+row-sum accumulate per pair); a trailing odd slot runs
    as a half tile, which also drains the pipeline faster
  - each core outputs per-tile partial sums; host sums and divides by B
"""

import numpy as np

import concourse.bacc as bacc
import concourse.bass as bass
import concourse.mybir as mybir
import concourse.tile as tile
from concourse.bass_utils import run_bass_kernel_spmd

B, D, C = 131072, 256, 1000
N_CORES = 8
P = 128   # SBUF partitions
S = 16    # rows per block (one label per block)

_nc_cache = {}


def _build(J):
    """Per-core graph for J block-slots per partition (J*8 rows each)."""
    if J in _nc_cache:
        return _nc_cache[J]
    splits = [1] * J
    acc_cols = sum(splits)

    nc = bacc.Bacc()
    feats = nc.declare_dram_parameter(
        "features", [J * P * S, D], mybir.dt.float32, isOutput=False
    )
    labels = nc.declare_dram_parameter("labels", [P, J], mybir.dt.int32, isOutput=False)
    centers = nc.declare_dram_parameter(
        "centers", [C, D], mybir.dt.float32, isOutput=False
    )
    out = nc.declare_dram_parameter(
        "out", [P, acc_cols], mybir.dt.float32, isOutput=True
    )

    # block-slot j, partition p, slot s <- feature row (j*128 + p)*8 + s
    fview = feats[:].rearrange("(j p s) d -> p j s d", p=P, s=S)

    with tile.TileContext(nc) as tc:
        with (
            tc.tile_pool(name="lab", bufs=1) as lab_pool,
            tc.tile_pool(name="f", bufs=4) as f_pool,
            tc.tile_pool(name="c", bufs=4) as c_pool,
            tc.tile_pool(name="acc", bufs=1) as acc_pool,
        ):
            lab = lab_pool.tile([P, J], mybir.dt.int32)
            nc.sync.dma_start(out=lab[:], in_=labels[:])
            acc = acc_pool.tile([P, acc_cols], mybir.dt.float32)
            col = 0
            for t in range(J):
                H = splits[t]
                SH = S // H
                f_t = f_pool.tile([P, S * D], mybir.dt.float32, tag="f")
                for h in range(H):
                    nc.sync.dma_start(
                        out=f_t[:, h * SH * D : (h + 1) * SH * D].rearrange(
                            "p (s d) -> p s d", s=SH
                        ),
                        in_=fview[:, t, h * SH : (h + 1) * SH, :],
                    )
                c_s = c_pool.tile([P, D], mybir.dt.float32, tag="c")
                nc.gpsimd.indirect_dma_start(
                    out=c_s[:],
                    out_offset=None,
                    in_=centers[:],
                    in_offset=bass.IndirectOffsetOnAxis(ap=lab[:, t : t + 1], axis=0),
                )
                c_b = (
                    c_s[:]
                    .rearrange("p (s d) -> p s d", s=1)
                    .to_broadcast([P, SH, D])
                )
                for h in range(H):
                    fh = f_t[:, h * SH * D : (h + 1) * SH * D]
                    nc.vector.tensor_tensor(
                        out=fh.rearrange("p (s d) -> p s d", s=SH),
                        in0=fh.rearrange("p (s d) -> p s d", s=SH),
                        in1=c_b,
                        op=mybir.AluOpType.subtract,
                    )
                    nc.scalar.activation(
                        out=fh,
                        in_=fh,
                        func=mybir.ActivationFunctionType.Square,
                        accum_out=acc[:, col : col + 1],
                    )
                    col += 1
            nc.sync.dma_start(out=out[:], in_=acc[:])
    nc.finalize()
    _nc_cache[J] = nc
    return nc


def _prepare(features, centers, labels):
    """Sort rows by label into padded S-row blocks; returns per-core maps + J."""
    features = np.ascontiguousarray(np.asarray(features), dtype=np.float32)
    centers = np.ascontiguousarray(np.asarray(centers), dtype=np.float32)
    labels = np.asarray(labels).astype(np.int32)

    counts = np.bincount(labels, minlength=C)          # [C]
    nblocks = -(-counts // S)                          # ceil(n_c / S) per class
    nb = int(nblocks.sum())
    group = N_CORES * P                                # blocks per slot across chip
    nb_pad = -(-nb // group) * group
    J = nb_pad // group                                # block-slots per partition

    # block labels, in sorted-class order; pad blocks use class 0
    block_labels = np.zeros(nb_pad, dtype=np.int32)
    block_labels[:nb] = np.repeat(np.arange(C, dtype=np.int32), nblocks)

    # every padded slot starts as its block's center row -> contributes 0
    fpad = centers[block_labels].repeat(S, axis=0).reshape(nb_pad * S, D)

    # scatter the real rows into their slots
    order = np.argsort(labels)
    labels_sorted = labels[order]
    class_row_start = np.concatenate(([0], np.cumsum(counts)[:-1]))
    class_slot_start = S * np.concatenate(([0], np.cumsum(nblocks)[:-1]))
    rank = np.arange(B) - class_row_start[labels_sorted]
    dst = class_slot_start[labels_sorted] + rank
    fpad[dst] = features[order]

    rows_core = J * P * S
    maps = []
    for k in range(N_CORES):
        fs = fpad[k * rows_core : (k + 1) * rows_core]
        # labW[p, j] = block_labels[(k*J + j)*128 + p]
        lw = np.ascontiguousarray(
            block_labels[k * J * P : (k + 1) * J * P].reshape(J, P).T
        )
        maps.append({"features": fs, "labels": lw, "centers": centers})
    return maps, J


def run(features, centers, labels, trace=False):
    """Run on 8 cores; returns (loss_scalar, BassKernelResults)."""
    maps, J = _prepare(features, centers, labels)
    nc = _build(J)
    res = run_bass_kernel_spmd(
        nc, maps, core_ids=list(range(N_CORES)), trace=trace
    )
    total = 0.0
    for r in res.results:
        total += float(np.asarray(r["out"]).astype(np.float64).sum())
    return np.float32(total / B), res


def kernel(features, centers, labels):
    loss, _ = run(features, centers, labels)
    return loss
